# revision 25
# baseline (speedup 1.0000x reference)
"""Trainium2 Bass kernel for nn_DiscontinuedGRU (bidirectional masked GRU).

Math (per direction, torch GRUCell):
    gx = x @ Wih.T + bih ; gh = h @ Whh.T + bhh
    r = sigmoid(gxr+ghr); z = sigmoid(gxz+ghz); n = tanh(gxn + r*ghn)
    h' = (1-z)*n + z*h_in,  h_in = 0 whenever the mask resets (else prev h)

Key idea: the Bernoulli(0.5) reset mask D chops every (batch, direction)
sequence into independent segments (mean length 2, max ~20).  Host code
derives the segments from D, sorts them by length descending, and the
device processes "round k" = the k-th element of every segment as one large
batched GRU-cell evaluation (big matmuls + gates).  Sequential depth
collapses from 2048 steps to KDEV(=6) device rounds; segments longer than
KDEV (P ~ 2^-k, <3% of positions) are finished vectorized on the host
during output assembly, seeded by the device h-states already present in
the assembled output.  Batch is sharded 8 ways (8 batch/core); each core
runs both directions for its slice.

Device layout: feature-on-partition (H=256 -> 2 chunks of 128).  x rows
are pre-gathered on the host into round-compacted matmul-rhs order and
streamed in via chunked DMA.  Round k's h lives in an SBUF buffer (A/B
alternating per round); each finished round is dumped densely (bf16,
feature-major) and the host inverse-permutes into the (SEQ,B,2H) output.

Per 256-position compute chunk the engines pipeline as: PE x-matmuls then
h-matmuls into double-buffered PSUM -> Act sigmoid (per-gate f32 bias) ->
DVE t1/nadd (scalar_tensor_tensor, releases the PSUM banks) -> Act tanh
-> DVE combine.  Chunks of the two directions are emitted round-robin,
stage-zipped in pairs, with tanh+combine lagged one pair so the act queue
never blocks on an in-flight DVE result.  PSUM accumulation obeys the
bank-granular start semantics (start=True clears the whole bank's
has_written bits -> exactly one start per bank per chunk).
"""

import os
import sys
from contextlib import ExitStack

for _p in ("/opt/trn_rl_repo", "/root/.axon_site/_ro/trn_rl_repo"):
    if os.path.isdir(_p) and _p not in sys.path:
        sys.path.insert(0, _p)

import numpy as np
import ml_dtypes

import concourse.bass as bass
import concourse.tile as tile
from concourse import bacc, mybir
from concourse.bass_utils import run_bass_kernel_spmd


BF = np.float16          # device 16-bit storage dtype (fp16: e5m10)
F8NP = ml_dtypes.float8_e4m3   # TRN-style e4m3 (max 240)
F32 = mybir.dt.float32
BF16 = mybir.dt.float16  # all values fit fp16 range; 8x finer mantissa
FP8 = mybir.dt.float8e4
WSC = 64.0               # fp8 weight pre-scale (dodges e4m3 subnormals);
                         # every gate matmul is x64, activations scale=1/64

SEQ, B, I, H = 2048, 64, 256, 256
NCORES = 8
CHUNK = int(os.environ.get("GRU_CHUNK", "256"))   # compute chunk
PBUFS = int(os.environ.get("GRU_PBUFS", "2"))     # PSUM double-buffering
BMM_N = int(os.environ.get("GRU_BMM_N", "0"))   # nk <= this -> rz bias matmuls
PAIR = os.environ.get("GRU_PAIR", "1") != "0"     # stage-zip chunk pairs
LAGD = int(os.environ.get("GRU_LAGD", "1"))       # pair-lag depth for tanh+comb
NAPOOL = int(os.environ.get("GRU_NAPOOL", "0"))   # Pool can't read PSUM: keep 0
GCH = int(os.environ.get("GRU_GCH", "2048"))
KDEV = int(os.environ.get("GRU_KDEV", "6"))  # device rounds; rest on host
# fp8 mode: 0 = all-fp16; 1 = r-gate plain fp8 (x+h, DoubleRow);
# 2 = G4: r plain fp8, z x-hilo + h plain fp8, n x-hilo fp8 + h fp16
# (numpy-simulated rel-err: 0=0.0019, 1=0.0072, 2=0.0094; budget 2e-2)
FP8MODE = int(os.environ.get("GRU_FP8", "2"))
# bias matmul for k>0 too (1-instr sigmoid); PE has slack in fp8 modes only
BMMALL = os.environ.get("GRU_BMMALL",
                        "1" if FP8MODE else "0") != "0"
H8POOL = os.environ.get("GRU_H8POOL", "1") != "0"  # h->fp8 copy on Pool
# h-n subtract on Pool: pays off once DVE (not PE) is the bottleneck
POOLDT = os.environ.get("GRU_POOLDT",
                        "1" if FP8MODE else "0") != "0"
# bhh_n rides a ones-matmul into P_nh (k>0) -> t1 is one tensor_tensor
NHB = os.environ.get("GRU_NHB", "1" if FP8MODE else "0") != "0"

Sigmoid = mybir.ActivationFunctionType.Sigmoid
Tanh = mybir.ActivationFunctionType.Tanh
DR = mybir.MatmulPerfMode.DoubleRow
MULT = mybir.AluOpType.mult
ADD = mybir.AluOpType.add
SUB = mybir.AluOpType.subtract


# ----------------------------------------------------------------- planning

def _segments(Dloc, reverse):
    """Segments of one direction for one core's (T, BL) mask slice,
    sorted by length descending."""
    T, BL = Dloc.shape
    segs = []
    for b in range(BL):
        if not reverse:
            m = (Dloc[:, b] == 1).copy()
            m[0] = True
            starts = np.flatnonzero(m)
            lens = np.diff(np.append(starts, T))
        else:
            m = np.zeros(T, dtype=bool)
            m[:-1] = Dloc[1:, b] == 1
            m[T - 1] = True
            starts = np.flatnonzero(m)
            lens = np.diff(np.concatenate([[-1], starts]))
        for s, L in zip(starts.tolist(), lens.tolist()):
            segs.append((int(L), int(s), int(b)))
    segs.sort(key=lambda x: -x[0])
    return segs


def _round_up(v, m):
    return (v + m - 1) // m * m


class Plan:
    pass


def make_plan(D, T=SEQ, ncores=NCORES):
    """Global round structure + per-core gather indices & output col maps."""
    align = int(os.environ.get("GRU_ALIGN", "32"))
    BL = D.shape[1] // ncores
    p = Plan()
    p.T, p.BL, p.ncores = T, BL, ncores
    p.core_segs = []
    for c in range(ncores):
        Dloc = np.asarray(D[:, c * BL:(c + 1) * BL])
        p.core_segs.append([_segments(Dloc, False), _segments(Dloc, True)])

    p.K = [0, 0]
    p.Kfull = [0, 0]
    p.Nk = [[], []]
    p.offs = [[], []]
    p.PT = [0, 0]
    for d in range(2):
        p.Kfull[d] = max(segs[d][0][0] for segs in p.core_segs)
        # segments longer than KDEV are rare (P(L>k) ~ 2^-k); their tail
        # steps run vectorized on the host from the assembled output
        K = min(p.Kfull[d], max(KDEV, 2))
        p.K[d] = K
        off = 0
        for k in range(K):
            n_glob = max(sum(1 for L, _, _ in segs[d] if L > k)
                         for segs in p.core_segs)
            # full-CHUNK alignment when cheap (avoids ragged tail chunks
            # that hiccup the pipeline), else 32-granular
            nk = _round_up(max(n_glob, 32), 32)
            if align > 32 and _round_up(nk, align) - nk <= 64:
                nk = _round_up(nk, align)
            if k > 0:
                nk = min(nk, p.Nk[d][k - 1])
            p.Nk[d].append(nk)
            p.offs[d].append(off)
            off += nk
        p.PT[d] = off

    p.xrows = []    # [core] -> (PT0+PT1,) int64 permuted X-row list
    p.col2row = []  # [core][dir] -> (PT,) int32, -1 = padding
    p.xc0 = [0, p.PT[0]]
    for c in range(ncores):
        rows_all = np.zeros(p.PT[0] + p.PT[1], dtype=np.int64)
        maps = []
        for d in range(2):
            segs = p.core_segs[c][d]
            c2r = np.full(p.PT[d], -1, dtype=np.int32)
            for k in range(p.K[d]):
                nreal = sum(1 for L, _, _ in segs if L > k)
                for j in range(nreal):
                    L, s, b = segs[j]
                    t = s + k if d == 0 else s - k
                    rows_all[p.xc0[d] + p.offs[d][k] + j] = t * BL + b
                    c2r[p.offs[d][k] + j] = t * BL + b
            maps.append(c2r)
        p.xrows.append(rows_all)
        p.col2row.append(maps)
    return p


# ----------------------------------------------------------------- builder

def build_program(p):
    """Emit the SPMD Bass/Tile program for plan p.

    Per chunk of n round-positions (dir d, round k):
      PE:  x-matmuls first (rz 8, ni 4), then h-matmuls (rz 8, nh 4)
      Act: sigmoid rz (4 instrs, per-gate f32 bias; or 1 instr when the
           round is small enough that rz bias rides K=1 bias matmuls)
      DVE: t1 = (P_nh + bhh_n) * r; nadd = (P_ni + bih_n) + t1
      Act: nt = tanh(nadd)
      DVE: h' = nt + z * (h - nt)
    No identity matmul (the old PE round-trip for r*ghn serialized the
    PE against Act+DVE every chunk), no bias matmuls on big rounds.
    PSUM pools are double-buffered (CHUNK=256 fits 2x(2+1+1) banks);
    chunks of the two directions are emitted round-robin and stage-zipped
    in pairs so Act/DVE work of chunk i overlaps PE work of chunk i+1.
    """
    T, BL = p.T, p.BL
    XB = 3 if LAGD >= 2 else 4          # SBUF budget: deeper lag needs
    SB = 2 * (LAGD + 1) if LAGD else 3  # more rz/na bufs, shallower xpool
    nc = bacc.Bacc("TRN2", target_bir_lowering=False, debug=False,
                   num_devices=p.ncores)

    PTsum = p.PT[0] + p.PT[1]
    fm = FP8MODE
    X_d = X8h_d = X8l_d = None
    if fm < 2:
        X_d = nc.dram_tensor("xg", [128, 2, PTsum], BF16,
                             kind="ExternalInput")
    if fm >= 1:
        X8h_d = nc.dram_tensor("xg8h", [128, 2, PTsum], FP8,
                               kind="ExternalInput")
    if fm == 2:
        X8l_d = nc.dram_tensor("xg8l", [128, 2, PTsum], FP8,
                               kind="ExternalInput")
    # fp16 weights: full ih/hh in modes 0-1; mode 2 keeps only hh n-rows
    w_d = [{} for _ in range(2)]
    for d in range(2):
        if fm < 2:
            w_d[d]["ih16"] = nc.dram_tensor(f"wih{d}", [2, 128, 3 * H], BF16,
                                            kind="ExternalInput")
        if fm < 2:
            w_d[d]["hh16"] = nc.dram_tensor(f"whh{d}", [2, 128, 3 * H], BF16,
                                            kind="ExternalInput")
        else:
            w_d[d]["hh16n"] = nc.dram_tensor(f"whhn{d}", [2, 128, H], BF16,
                                             kind="ExternalInput")
        if fm >= 1:
            w_d[d]["ih8h"] = nc.dram_tensor(
                f"w8ih{d}", [2, 128, 3 * H if fm == 2 else H], FP8,
                kind="ExternalInput")
            w_d[d]["hh8h"] = nc.dram_tensor(
                f"w8hh{d}", [2, 128, 2 * H if fm == 2 else H], FP8,
                kind="ExternalInput")
        if fm == 2:
            w_d[d]["ih8l"] = nc.dram_tensor(f"w8ihl{d}", [2, 128, 2 * H], FP8,
                                            kind="ExternalInput")
    bias_d = nc.dram_tensor("biasbf", [1, 2048], BF16, kind="ExternalInput")
    b32_d = nc.dram_tensor("bias32", [128, 16], F32, kind="ExternalInput")
    ones_d = nc.dram_tensor("onesbf", [1, CHUNK], BF16, kind="ExternalInput")
    hout_d = [nc.dram_tensor(f"h{'fb'[d]}", [128, 2, p.PT[d]], BF16,
                             kind="ExternalOutput") for d in range(2)]

    with tile.TileContext(nc) as tc, ExitStack() as ctx:
        cpool = ctx.enter_context(tc.tile_pool(name="consts", bufs=1))
        wt = [{} for _ in range(2)]
        wcols = {"ih16": 3 * H, "hh16": 3 * H, "hh16n": H,
                 "ih8h": 3 * H if fm == 2 else H,
                 "hh8h": 2 * H if fm == 2 else H, "ih8l": 2 * H}
        for d in range(2):
            for nm, dd in w_d[d].items():
                t = cpool.tile([128, 2, wcols[nm]],
                               BF16 if "16" in nm else FP8,
                               tag=f"w{nm}{d}", name=f"w{nm}{d}")
                for kk in range(2):
                    nc.sync.dma_start(t[:, kk, :], dd.ap()[kk])
                wt[d][nm] = t
        bias_t = cpool.tile([1, 2048], BF16, tag="biasbf")
        nc.sync.dma_start(bias_t[:], bias_d.ap())
        b32_t = cpool.tile([128, 16], F32, tag="bias32")
        nc.sync.dma_start(b32_t[:], b32_d.ap())
        ones_t = cpool.tile([1, CHUNK], BF16, tag="onesbf")
        nc.sync.dma_start(ones_t[:], ones_d.ap())

        # h buffers: slot A holds even rounds (sized for round 0),
        # slot B odd rounds
        hszA = [p.Nk[d][0] for d in range(2)]
        hszB = [p.Nk[d][1] if p.K[d] > 1 else 128 for d in range(2)]
        hbA = [cpool.tile([128, 2, hszA[d]], BF16, tag=f"hA{d}",
                          name=f"hA{d}") for d in range(2)]
        hbB = [cpool.tile([128, 2, hszB[d]], BF16, tag=f"hB{d}",
                          name=f"hB{d}") for d in range(2)]
        # fp8 copies of h for the r/z-gate h-matmuls (written only for
        # columns that round k+1 will actually read: Nk[k+1] ~ Nk[k]/2)
        h8A = h8B = None
        if fm >= 1:
            h8A = [cpool.tile([128, 2, p.Nk[d][1] if p.K[d] > 1 else 32],
                              FP8, tag=f"h8A{d}", name=f"h8A{d}")
                   for d in range(2)]
            h8B = [cpool.tile([128, 2, p.Nk[d][2] if p.K[d] > 2 else 32],
                              FP8, tag=f"h8B{d}", name=f"h8B{d}")
                   for d in range(2)]

        big = CHUNK >= 512   # SBUF budget: shrink pools at CHUNK=512
        xpool = ctx.enter_context(tc.tile_pool(name="xg", bufs=3 if big else XB))
        rzpool = ctx.enter_context(tc.tile_pool(name="rz", bufs=3 if big else SB))
        t1pool = ctx.enter_context(tc.tile_pool(name="t1", bufs=3))
        napool = ctx.enter_context(tc.tile_pool(name="na", bufs=3 if big else SB))
        npool = ctx.enter_context(tc.tile_pool(name="nt", bufs=3))
        dpool = ctx.enter_context(tc.tile_pool(name="dt", bufs=2))
        epool = ctx.enter_context(tc.tile_pool(name="et", bufs=2))
        prz = ctx.enter_context(tc.tile_pool(name="prz", bufs=PBUFS,
                                             space="PSUM"))
        pnh = ctx.enter_context(tc.tile_pool(name="pnh", bufs=PBUFS,
                                             space="PSUM"))
        pni = ctx.enter_context(tc.tile_pool(name="pni", bufs=PBUFS,
                                             space="PSUM"))

        def chunk_stages(d, k, hprev, hnew, xt, xoff, j0, n, bmm, dma=None,
                         h8prev=None, h8new=None, h8cols=0):
            """Stage closures for positions [j0, j0+n) of round k, dir d."""
            bb = 1024 * d
            st = {}

            def s_mm():
                if dma is not None:
                    dma()
                P_rz = prz.tile([128, 4, CHUNK], F32, tag="prz", name="P_rz")
                P_ni = pni.tile([128, 2, CHUNK], F32, tag="pni", name="P_ni")
                st["P_rz"], st["P_ni"] = P_rz, P_ni
                xsl = slice(j0 - xoff, j0 - xoff + n)
                hs = hprev[:, :, j0:j0 + n] if k > 0 else None
                h8s = h8prev[:, :, j0:j0 + n] if (k > 0 and fm >= 1) else None
                st["hs"] = hs
                P_nh = None
                if k > 0:
                    P_nh = pnh.tile([128, 2, CHUNK], F32, tag="pnh",
                                    name="P_nh")
                    st["P_nh"] = P_nh
                # start=True clears the has_written bits of the WHOLE PSUM
                # bank, so with two 256-col gate slots per bank it must fire
                # only on the first matmul touching the bank; per-element
                # has_written then makes each slot's first write an
                # overwrite and later writes accumulate.  One group per
                # bank: stop only on the bank's last write.
                spb = 2 * 256 // CHUNK
                plan = []   # (ps_ap, bank, lhsT, rhs, perf_mode)

                def rz_x(m):
                    ps, bank = P_rz[:, m, 0:n], ("rz", m // spb)
                    gate_fp8 = (fm >= 1 and m < 2) or fm == 2
                    if not gate_fp8:
                        for kk in range(2):
                            plan.append((ps, bank,
                                         wt[d]["ih16"][:, kk,
                                                       m * 128:m * 128 + 128],
                                         xt["16"][:, kk, xsl], None))
                        return
                    wh = wt[d]["ih8h"][:, :, m * 128:m * 128 + 128]
                    plan.append((ps, bank, wh, xt["8h"][:, :, xsl], DR))
                    if fm == 2 and m >= 2:   # z: hi-lo x
                        wl = wt[d]["ih8l"][:, :, (m - 2) * 128:
                                           (m - 2) * 128 + 128]
                        plan.append((ps, bank, wh, xt["8l"][:, :, xsl], DR))
                        plan.append((ps, bank, wl, xt["8h"][:, :, xsl], DR))

                def rz_h(m):
                    ps, bank = P_rz[:, m, 0:n], ("rz", m // spb)
                    gate_fp8 = fm >= 1 and (m < 2 or fm == 2)
                    if gate_fp8:
                        plan.append((ps, bank,
                                     wt[d]["hh8h"][:, :, m * 128:m * 128 + 128],
                                     h8s, DR))
                    else:
                        for kk in range(2):
                            plan.append((ps, bank,
                                         wt[d]["hh16"][:, kk,
                                                       m * 128:m * 128 + 128],
                                         hs[:, kk, :], None))

                def ni_x(mi):
                    m = 4 + mi
                    ps, bank = P_ni[:, mi, 0:n], ("ni", mi // spb)
                    if fm == 2:   # n: hi-lo x
                        wh = wt[d]["ih8h"][:, :, m * 128:m * 128 + 128]
                        wl = wt[d]["ih8l"][:, :, (m - 2) * 128:
                                           (m - 2) * 128 + 128]
                        plan.append((ps, bank, wh, xt["8h"][:, :, xsl], DR))
                        plan.append((ps, bank, wh, xt["8l"][:, :, xsl], DR))
                        plan.append((ps, bank, wl, xt["8h"][:, :, xsl], DR))
                    else:
                        for kk in range(2):
                            plan.append((ps, bank,
                                         wt[d]["ih16"][:, kk,
                                                       m * 128:m * 128 + 128],
                                         xt["16"][:, kk, xsl], None))

                def nh_h(mi):
                    m = 4 + mi
                    ps, bank = P_nh[:, mi, 0:n], ("nh", mi // spb)
                    if NHB:   # bhh_n bias -> t1 becomes one tensor_tensor
                        plan.append((ps, bank,
                                     bias_t[0:1, bb + 512 + mi * 128:
                                            bb + 512 + mi * 128 + 128],
                                     ones_t[0:1, 0:n], None))
                    wtile = wt[d]["hh16n" if fm == 2 else "hh16"]
                    moff = mi if fm == 2 else m
                    for kk in range(2):
                        plan.append((ps, bank,
                                     wtile[:, kk, moff * 128:moff * 128 + 128],
                                     hs[:, kk, :], None))

                if bmm:
                    for m in range(4):
                        plan.append((P_rz[:, m, 0:n], ("rz", m // spb),
                                     bias_t[0:1, bb + m * 128:
                                            bb + m * 128 + 128],
                                     ones_t[0:1, 0:n], None))
                # rz matmuls complete first (x then h) so the sigmoid ->
                # t1 -> nadd chain starts as early as possible; n-gate
                # matmuls follow
                for m in range(4):
                    rz_x(m)
                if k > 0:
                    for m in range(4):
                        rz_h(m)
                for mi in range(2):
                    ni_x(mi)
                if k > 0:
                    for mi in range(2):
                        nh_h(mi)

                tot = {}
                for _, bank, _, _, _ in plan:
                    tot[bank] = tot.get(bank, 0) + 1
                left = dict(tot)
                for ps, bank, lhsT, rhs, pm in plan:
                    first = left[bank] == tot[bank]
                    left[bank] -= 1
                    nc.tensor.matmul(ps, lhsT, rhs, start=first,
                                     stop=left[bank] == 0, perf_mode=pm)

            def s_sig():
                rz = rzpool.tile([128, 4, n], BF16, tag="rz", name="rz")
                if bmm:
                    nc.scalar.activation(rz[:], st["P_rz"][:, :, 0:n], Sigmoid,
                                         scale=1.0 / WSC)
                else:
                    for m in range(4):
                        nc.scalar.activation(rz[:, m, :], st["P_rz"][:, m, 0:n],
                                             Sigmoid,
                                             bias=b32_t[:, 8 * d + m:
                                                        8 * d + m + 1],
                                             scale=1.0 / WSC)
                st["r"], st["z"] = rz[:, 0:2, :], rz[:, 2:4, :]

            def s_t1na():
                # t1 then nadd back-to-back on DVE: nadd's read releases
                # P_ni for the chunk-after-next's matmuls (WAR), so it must
                # not queue behind the pair-sibling's ops.
                r = st["r"]
                t1 = t1pool.tile([128, 2, n], F32 if k > 0 else BF16,
                                 tag="t1" if k > 0 else "t1z", name="t1")
                st["t1"] = t1
                if k > 0 and NHB:
                    # bhh_n already in P_nh (ones-matmul): one tt instr
                    nc.vector.tensor_tensor(t1[:], st["P_nh"][:, :, 0:n],
                                            r, MULT)
                elif k > 0:
                    # t1 = (ghn + bhh_n) * r   (bhh_n per-partition)
                    for mm in range(2):
                        nc.vector.scalar_tensor_tensor(
                            t1[:, mm, :], st["P_nh"][:, mm, 0:n],
                            b32_t[:, 8 * d + 6 + mm:8 * d + 7 + mm],
                            r[:, mm, :], ADD, MULT)
                else:
                    for mm in range(2):
                        nc.vector.tensor_scalar_mul(
                            t1[:, mm, :], r[:, mm, :],
                            b32_t[:, 8 * d + 6 + mm:8 * d + 7 + mm])
                na = napool.tile([128, 2, n], F32, tag="na", name="na")
                st["na"] = na
                # nadd = (gxn + bih_n) + t1; on Pool (idle engine) when
                # NAPOOL -- DVE is ~88% busy, Pool ~0%.
                for mm in range(2):
                    eng = nc.gpsimd if NAPOOL > mm else nc.vector
                    eng.scalar_tensor_tensor(
                        na[:, mm, :], st["P_ni"][:, mm, 0:n],
                        b32_t[:, 8 * d + 4 + mm:8 * d + 5 + mm],
                        st["t1"][:, mm, :], ADD, ADD)

            def s_tanh():
                nt = npool.tile([128, 2, n], BF16, tag="nt", name="nt")
                st["nt"] = nt
                nc.scalar.activation(nt[:], st["na"][:], Tanh,
                                     scale=1.0 / WSC)

            def s_comb():
                nt, z = st["nt"], st["z"]
                hd = hnew[:, :, j0:j0 + n]
                et = epool.tile([128, 2, n], BF16, tag="et", name="et")
                eng = nc.gpsimd if (POOLDT and n == CHUNK) else nc.vector
                if k > 0:
                    dt = dpool.tile([128, 2, n], BF16, tag="dt", name="dt")
                    eng.tensor_tensor(dt[:], st["hs"], nt[:], SUB)
                    nc.vector.tensor_tensor(et[:], z, dt[:], MULT)
                    nc.vector.tensor_tensor(hd, nt[:], et[:], ADD)
                else:
                    nc.vector.tensor_tensor(et[:], z, nt[:], MULT)
                    nc.vector.tensor_tensor(hd, nt[:], et[:], SUB)
                if h8cols > j0:
                    # fp8 copy of h' for the next round's r/z h-matmuls;
                    # only the columns round k+1 will read.
                    n8 = min(n, h8cols - j0)
                    e8 = nc.gpsimd if H8POOL else nc.vector
                    e8.tensor_copy(h8new[:, :, j0:j0 + n8],
                                   hnew[:, :, j0:j0 + n8])

            return [s_mm, s_sig, s_t1na, s_tanh, s_comb]

        for _rep in range(int(os.environ.get("GRU_REPEAT", "1"))):
            maxK = max(p.K)
            for k in range(maxK):
                per_dir = []        # [d] -> list of stage-lists
                for d in range(2):
                    if k >= p.K[d]:
                        continue
                    hprev = (hbB[d] if k % 2 == 0 else hbA[d]) if k else None
                    hnew = hbA[d] if k % 2 == 0 else hbB[d]
                    h8prev = h8new = None
                    h8cols = 0
                    if fm >= 1:
                        h8prev = (h8B[d] if k % 2 == 0 else h8A[d]) \
                            if k else None
                        h8new = h8A[d] if k % 2 == 0 else h8B[d]
                        if k + 1 < p.K[d]:
                            h8cols = p.Nk[d][k + 1]
                    nk = p.Nk[d][k]
                    # rz-bias strategy: K=1 bias matmuls + single sigmoid
                    # where PE has slack (round 0: no h-matmuls) or the round
                    # is small (fewer act instrs shortens the serial chain);
                    # per-gate activation bias where PE is the bottleneck.
                    bmm = k == 0 or BMMALL or nk <= BMM_N
                    chunks = []
                    for g0 in range(0, nk, GCH):
                        gn = min(GCH, nk - g0)
                        a0 = p.xc0[d] + p.offs[d][k] + g0
                        xt = {}
                        if fm < 2:
                            xt["16"] = xpool.tile([128, 2, gn], BF16,
                                                  tag="xg", name="xt")
                        if fm >= 1:
                            xt["8h"] = xpool.tile([128, 2, gn], FP8,
                                                  tag="x8h", name="xt8h")
                        if fm == 2:
                            xt["8l"] = xpool.tile([128, 2, gn], FP8,
                                                  tag="x8l", name="xt8l")

                        def dma(xt=xt, a0=a0, gn=gn):
                            if "16" in xt:
                                nc.sync.dma_start(
                                    xt["16"][:], X_d.ap()[:, :, a0:a0 + gn])
                            if "8h" in xt:
                                nc.sync.dma_start(
                                    xt["8h"][:], X8h_d.ap()[:, :, a0:a0 + gn])
                            if "8l" in xt:
                                nc.sync.dma_start(
                                    xt["8l"][:], X8l_d.ap()[:, :, a0:a0 + gn])
                        for j0 in range(g0, g0 + gn, CHUNK):
                            chunks.append(chunk_stages(
                                d, k, hprev, hnew, xt, g0, j0,
                                min(CHUNK, g0 + gn - j0), bmm,
                                dma=(dma if j0 == g0 else None),
                                h8prev=h8prev, h8new=h8new, h8cols=h8cols))
                    per_dir.append((d, hnew, nk, chunks))

                # round-robin chunks across the two directions
                order = []
                nmax = max(len(c) for _, _, _, c in per_dir)
                for i in range(nmax):
                    for _, _, _, chunks in per_dir:
                        if i < len(chunks):
                            order.append(chunks[i])
                # emit in pairs with stages zipped for cross-engine overlap;
                # tanh+comb lag one pair behind (software pipeline) so the
                # act engine's sigmoids are never queued behind a tanh that
                # waits on the just-issued DVE nadd.
                lag = LAGD if (PAIR and len(order) > 2 * LAGD) else 0
                pend = []   # queue of late-stage groups, one per pair
                i = 0
                while i < len(order):
                    pair = order[i:i + 2] if PAIR else order[i:i + 1]
                    early = [st[:3] for st in pair]
                    for stages in zip(*early) if len(pair) > 1 else \
                            [(s,) for s in early[0]]:
                        for s in stages:
                            s()
                    pend.append([st[3:] for st in pair])
                    while len(pend) > lag:
                        for st2 in pend.pop(0):
                            for s in st2:
                                s()
                    i += len(pair)
                while pend:
                    for st2 in pend.pop(0):
                        for s in st2:
                            s()
                for d, hnew, nk, _ in per_dir:
                    c0 = p.offs[d][k]
                    nc.sync.dma_start(hout_d[d].ap()[:, :, c0:c0 + nk],
                                      hnew[:, :, 0:nk])

    nc.compile()
    return nc


# ------------------------------------------------------------- host driver

def _shared_consts(Wih_f, Whh_f, bih_f, bhh_f, Wih_b, Whh_b, bih_b, bhh_b):
    """All gate weights pre-scaled by WSC=64 (fp16 exact; keeps fp8 weights
    out of the e4m3 subnormal range); sigmoid/tanh run with scale=1/64."""
    wb = {}
    for d, (Wih, Whh) in enumerate([(Wih_f, Whh_f), (Wih_b, Whh_b)]):
        WTi = np.ascontiguousarray(Wih.T).astype(np.float32) * WSC  # (I,3H)
        WTh = np.ascontiguousarray(Whh.T).astype(np.float32) * WSC
        if FP8MODE < 2:
            wb[f"wih{d}"] = np.ascontiguousarray(
                WTi.astype(BF).reshape(2, 128, 3 * H))
            wb[f"whh{d}"] = np.ascontiguousarray(
                WTh.astype(BF).reshape(2, 128, 3 * H))
        else:
            wb[f"whhn{d}"] = np.ascontiguousarray(
                WTh[:, 2 * H:].astype(BF).reshape(2, 128, H))
        if FP8MODE >= 1:
            ih_hi = WTi.astype(F8NP)
            hh_hi = WTh.astype(F8NP)
            if FP8MODE == 2:
                wb[f"w8ih{d}"] = np.ascontiguousarray(
                    ih_hi.reshape(2, 128, 3 * H))
                wb[f"w8hh{d}"] = np.ascontiguousarray(
                    hh_hi[:, :2 * H].reshape(2, 128, 2 * H))
                ih_lo = (WTi - ih_hi.astype(np.float32)).astype(F8NP)
                wb[f"w8ihl{d}"] = np.ascontiguousarray(
                    ih_lo[:, H:].reshape(2, 128, 2 * H))
            else:
                wb[f"w8ih{d}"] = np.ascontiguousarray(
                    ih_hi[:, :H].reshape(2, 128, H))
                wb[f"w8hh{d}"] = np.ascontiguousarray(
                    hh_hi[:, :H].reshape(2, 128, H))
    bias = np.zeros((1, 2048), dtype=BF)
    b32 = np.zeros((128, 16), dtype=np.float32)
    for d, (bih, bhh) in enumerate([(bih_f, bhh_f), (bih_b, bhh_b)]):
        # bmm bias rides the 64-scaled PSUM; per-gate b32 rz bias is applied
        # AFTER the 1/64 activation scale, so it stays unscaled.
        bias[0, 1024 * d:1024 * d + 512] = \
            ((bih[:512] + bhh[:512]) * WSC).astype(BF)
        bias[0, 1024 * d + 512:1024 * d + 768] = \
            (bhh[512:] * WSC).astype(BF)   # NHB: bhh_n via ones-matmul
        brz = (bih[:512] + bhh[:512]).astype(np.float32)
        for m in range(4):
            b32[:, 8 * d + m] = brz[m * 128:(m + 1) * 128]
        b32[:, 8 * d + 4] = bih[512:640] * WSC
        b32[:, 8 * d + 5] = bih[640:768] * WSC
        b32[:, 8 * d + 6] = bhh[512:640] * WSC
        b32[:, 8 * d + 7] = bhh[640:768] * WSC
    wb["biasbf"] = bias
    wb["bias32"] = b32
    wb["onesbf"] = np.ones((1, CHUNK), dtype=BF)
    return wb


def make_in_maps(p, X, wb):
    BL = p.BL
    in_maps = []
    for c in range(p.ncores):
        Xc = np.ascontiguousarray(
            X[:, c * BL:(c + 1) * BL, :]).reshape(p.T * BL, I)
        R = Xc[p.xrows[c]]                       # (PTsum, 256) permuted, f32
        m = {}

        def fold(a):   # (PTsum, 256) -> (128, 2, PTsum)
            return np.ascontiguousarray(
                a.reshape(-1, 2, 128).transpose(2, 1, 0))

        if FP8MODE < 2:
            m["xg"] = fold(R.astype(BF))
        if FP8MODE >= 1:
            hi = R.astype(F8NP)
            m["xg8h"] = fold(hi)
            if FP8MODE == 2:
                m["xg8l"] = fold((R - hi.astype(np.float32)).astype(F8NP))
        m.update(wb)
        in_maps.append(m)
    return in_maps


def assemble_output(p, results):
    """Inverse-permute per-core dumps into the full (T, B, 2H) output."""
    T, BL = p.T, p.BL
    out = np.empty((T, p.ncores * BL, 2 * H), dtype=np.float32)
    for c in range(p.ncores):
        for d in range(2):
            hT = np.asarray(results[c]["h" + "fb"[d]]).astype(np.float32)
            hfull = hT.transpose(1, 0, 2).reshape(2 * 128, p.PT[d])
            c2r = p.col2row[c][d]
            valid = c2r >= 0
            block = hfull[:, valid].T
            rows = c2r[valid]
            out[rows // BL, c * BL + rows % BL, d * H:(d + 1) * H] = block
    return out


def _host_tail(p, X, ws, out):
    """Finish segments longer than p.K[d] on the host.  P(L>k) ~ 2^-k so
    this touches only a few hundred positions; h at device round K-1 is
    already in the assembled output (the GRU output IS the hidden state)."""
    BL = p.BL

    def sig(v):
        return 1.0 / (1.0 + np.exp(-v))

    for d in range(2):
        K0, maxL = p.K[d], p.Kfull[d]
        if maxL <= K0:
            continue
        Wih, Whh, bih, bhh = ws[d]
        WihT = np.ascontiguousarray(Wih.T)
        WhhT = np.ascontiguousarray(Whh.T)
        act = []
        for c in range(p.ncores):
            for L, s, b in p.core_segs[c][d]:
                if L > K0:
                    act.append((c * BL + b, s, L))
        act.sort(key=lambda x: -x[2])
        bg = np.array([a[0] for a in act])
        ss = np.array([a[1] for a in act])
        Ls = np.array([a[2] for a in act])
        tprev = ss + (K0 - 1) if d == 0 else ss - (K0 - 1)
        h = out[tprev, bg, d * H:(d + 1) * H].astype(np.float32)
        for k in range(K0, maxL):
            nact = int((Ls > k).sum())   # sorted desc -> active is a prefix
            if nact == 0:
                break
            h, bg, ss = h[:nact], bg[:nact], ss[:nact]
            t = ss + k if d == 0 else ss - k
            gx = X[t, bg, :] @ WihT + bih
            gh = h @ WhhT + bhh
            r = sig(gx[:, 0:H] + gh[:, 0:H])
            z = sig(gx[:, H:2 * H] + gh[:, H:2 * H])
            n = np.tanh(gx[:, 2 * H:] + r * gh[:, 2 * H:])
            h = (1.0 - z) * n + z * h
            out[t, bg, d * H:(d + 1) * H] = h
    return out


def kernel(**inputs):
    X = np.asarray(inputs["X"], dtype=np.float32)
    D = np.asarray(inputs["D"])
    p = make_plan(D)
    fw = [np.asarray(inputs[k], dtype=np.float32) for k in
          ("Wih_f", "Whh_f", "bih_f", "bhh_f",
           "Wih_b", "Whh_b", "bih_b", "bhh_b")]
    wb = _shared_consts(*fw)
    nc = build_program(p)
    in_maps = make_in_maps(p, X, wb)
    res = run_bass_kernel_spmd(nc, in_maps, list(range(p.ncores)))
    out = assemble_output(p, res.results)
    return _host_tail(p, X, [fw[0:4], fw[4:8]], out)

